# revision 37
# baseline (speedup 1.0000x reference)
"""MuSc (Mutual Scoring) Trainium2 kernel.

Problem: nn_BatchMuSc — Z:[16,1369,1024] patch features, cls_tokens:[16,1024].
MSM: for each image i, per-patch score = mean of the 4 smallest per-image
min-distances (excluding self). Then image scores -> min-max norm -> MMO over
cls-token similarity.

Strategy (8 NeuronCores, pair-symmetric phase 1 + exact rescue phase 2):
  - The 120 unordered image pairs are split 15/core via a near-regular
    tournament on K16 (core c: query A=c vs refs c+1..c+8; query B=c+8 vs
    refs c+9..c+15). Each pair's distance block is computed ONCE; its row
    mins serve (A->ref) and its column mins serve (ref->A).
  - Cross terms are fp8e4 DoubleRow matmuls (K=256/instr, 2x fp16 rate).
    A 5th tiny fp16 K=4 matmul per chunk adds -0.5|q|^2 - 0.5|r|^2 so
    PSUM = -0.5 d^2; DVE then only runs the two irreducible passes
    (row max-reduce, column max-merge across query blocks). Column partials
    [128] finish on host (max over partitions, d^2 = -2v).
  - Phase-1 fp8 ranking noise (~0.06 in d) is erased by phase 2: the top-4
    candidate patches per image are re-scored at 2-term-fp16 precision
    (qh*rh + ql*rh), sharded 2 ref images/core; image score = max of exact
    candidate scores. Host does the tiny min-max norm + 16x16 MMO tail.
"""

import os
import numpy as np

N = 16            # images
L = 1369          # patches per image
C = 1024          # feature dim
NCORES = 8
LP = 1408         # padded patches (11 * 128)
NQB = 11          # query blocks of 128
KCH = 8           # contraction chunks of 128
CHUNKS = [(0, 512), (512, 512), (1024, 345)]   # 1369 real refs; pad cols excluded
PAD_VAL = np.float16(2.0)   # pad-row feature value; pad d^2 ~ |q|^2+4096-4*sum(q) >> real min
PAD_NORM = 4096.0           # C * PAD_VAL^2
BIG = 3.0e38

_CACHE = {}


def _build():
    import concourse.bacc as bacc
    import concourse.tile as tile
    from concourse import mybir

    f8 = mybir.dt.float8e4
    f32 = mybir.dt.float32
    Sqrt = mybir.ActivationFunctionType.Sqrt
    Alu = mybir.AluOpType
    AxX = mybir.AxisListType.X
    DR = mybir.MatmulPerfMode.DoubleRow

    nc = bacc.Bacc("TRN2", target_bir_lowering=False, debug=False)

    zt = nc.dram_tensor("zt", [N, 128, KCH, LP], f8, kind="ExternalInput").ap()
    nb = nc.dram_tensor("nb", [N, 128, LP], f32, kind="ExternalInput").ap()
    q2 = nc.dram_tensor("q2", [2, 128, NQB], f32, kind="ExternalInput").ap()
    out = nc.dram_tensor("scores", [2, 128, NQB], f32, kind="ExternalOutput").ap()

    with tile.TileContext(nc) as tc:
        with (
            tc.tile_pool(name="qpool", bufs=1) as qpool,
            tc.tile_pool(name="refpool", bufs=3) as refpool,
            tc.tile_pool(name="nbpool", bufs=3) as nbpool,
            tc.tile_pool(name="mpool", bufs=1) as mpool,
            tc.tile_pool(name="smpool", bufs=8) as smpool,
            tc.tile_pool(name="scrpool", bufs=6) as scrpool,
            tc.tile_pool(name="scorepool", bufs=1) as scorepool,
            tc.tile_pool(name="psum", bufs=7, space="PSUM") as psum,
        ):
            # resident tiles for the core's own 2 images: used as BOTH the
            # query lhsT (raw, un-scaled) and the ref tiles for positions 0/1
            qsb = []
            for i in range(2):
                t = qpool.tile([128, KCH, LP], f8, name=f"q{i}", tag=f"q{i}")
                nc.sync.dma_start(t[:], zt[i])
                qsb.append(t)
            q2sb = []
            for i in range(2):
                t = qpool.tile([128, NQB], f32, name=f"q2_{i}", tag=f"q2_{i}")
                nc.sync.dma_start(t[:], q2[i])
                q2sb.append(t)

            # persistent min accumulators m[i][qb] : [128, N] (d^2 - |q|^2 per ref pos)
            msb = [[mpool.tile([128, N], f32, name=f"m_{i}_{qb}", tag=f"m_{i}_{qb}") for qb in range(NQB)]
                   for i in range(2)]
            for i in range(2):
                for qb in range(NQB):
                    nc.vector.memset(msb[i][qb][:], -BIG)

            scoresb = [scorepool.tile([128, NQB], f32, name=f"sc{i}", tag=f"sc{i}") for i in range(2)]

            for t in [0] + list(range(2, N)) + [1]:
                if t < 2:
                    rsb = qsb[t]
                else:
                    rsb = refpool.tile([128, KCH, LP], f8, name="ref", tag="ref")
                    nc.sync.dma_start(rsb[:], zt[t])
                nbt = nbpool.tile([128, LP], f32, name="nbt", tag="nbt")
                nc.sync.dma_start(nbt[:], nb[t])

                for i in range(2):
                    if t == i:   # self image: skip
                        continue
                    for qb in range(NQB):
                        prev = None
                        for ci, (r0, w) in enumerate(CHUNKS):
                            pt = psum.tile([128, 512], f32, name="qr", tag="qr")
                            for kk in range(KCH // 2):
                                nc.tensor.matmul(
                                    pt[:, :w],
                                    lhsT=qsb[i][:, 2 * kk:2 * kk + 2,
                                                qb * 128:(qb + 1) * 128],
                                    rhs=rsb[:, 2 * kk:2 * kk + 2, r0:r0 + w],
                                    start=(kk == 0),
                                    stop=(kk == KCH // 2 - 1),
                                    perf_mode=DR,
                                )
                            scr = scrpool.tile([128, 512], f32, name="scr", tag="scr")
                            nc.vector.tensor_tensor(
                                scr[:, :w], pt[:, :w], nbt[:, r0:r0 + w],
                                op=Alu.subtract)
                            cm = smpool.tile([128, 1], f32, name="cmin", tag="cmin")
                            nc.vector.tensor_reduce(
                                cm[:], scr[:, :w], axis=AxX, op=Alu.max)
                            if ci == 0:
                                prev = cm
                            elif ci < len(CHUNKS) - 1:
                                nx = smpool.tile([128, 1], f32, name="nx", tag="nx")
                                nc.vector.tensor_tensor(
                                    nx[:], prev[:], cm[:], op=Alu.max)
                                prev = nx
                            else:
                                nc.vector.tensor_tensor(
                                    msb[i][qb][:, t:t + 1], prev[:], cm[:],
                                    op=Alu.max)

            # tail: per (img, qblock) extract 4 smallest, sqrt(x+|q|^2), mean
            for i in range(2):
                for qb in range(NQB):
                    m = msb[i][qb]
                    dsum = None
                    for it in range(4):
                        rmin = smpool.tile([128, 1], f32, name="rmin", tag="rmin")
                        nc.vector.tensor_reduce(rmin[:], m[:], axis=AxX, op=Alu.max)
                        if it < 3:
                            mask = smpool.tile([128, N], f32, name="mask", tag="mask")
                            nc.vector.tensor_scalar(
                                out=mask[:], in0=m[:],
                                scalar1=rmin[:], scalar2=-BIG,
                                op0=Alu.is_equal, op1=Alu.mult,
                            )
                            nc.vector.tensor_tensor(m[:], m[:], mask[:], op=Alu.add)
                        d = smpool.tile([128, 1], f32, name="dist", tag="dist")
                        nc.scalar.activation(d[:], rmin[:], Sqrt,
                                             bias=q2sb[i][:, qb:qb + 1], scale=-2.0)
                        if dsum is None:
                            dsum = d
                        else:
                            s = smpool.tile([128, 1], f32, name="dsum", tag="dsum")
                            nc.vector.tensor_add(s[:], dsum[:], d[:])
                            dsum = s
                    nc.vector.tensor_scalar_mul(
                        scoresb[i][:, qb:qb + 1], dsum[:], 0.25)

            for i in range(2):
                nc.sync.dma_start(out[i], scoresb[i][:])
    nc.compile()
    return nc


def _pairs_for_core(c):
    """15 unordered pairs per core via a near-regular tournament on K16:
    query A = c with refs c+1..c+8 (mod 16); query B = c+8 with refs
    c+9..c+15 (mod 16). Over c=0..7 every unordered pair appears once."""
    pa = [(c, (c + 1 + j) % N) for j in range(8)]
    pb = [((c + 8) % N, (c + 9 + j) % N) for j in range(7)]
    return pa + pb


def _build3():
    """Symmetric-pair MSM: each core computes 15 full cross-distance blocks
    D(q in A/B, r in ref) once. A 5th fp16 K=4 matmul per chunk folds BOTH
    norms into PSUM (lhsT rows = qn_hi, qn_lo, 1, 1; rhs rows = 1, 1,
    nbneg_hi, nbneg_lo), so PSUM = q.r - 0.5|q|^2 - 0.5|r|^2 = -0.5 d^2.
    DVE then only does the two irreducible passes per chunk:
      row: tensor_reduce max over refs -> mrow[:, qb] (chained over chunks)
      col: tensor_tensor max-merge across the 11 query blocks -> macc fp16
    Host finishes: d^2 = -2 * value in both directions (norms cancel).
    Cross matmuls are fp8e4 DoubleRow (K=256 per instruction)."""
    import concourse.bacc as bacc
    import concourse.tile as tile
    from concourse import mybir

    f8 = mybir.dt.float8e4
    f16 = mybir.dt.float16
    f32 = mybir.dt.float32
    Alu = mybir.AluOpType
    AxX = mybir.AxisListType.X
    DR = mybir.MatmulPerfMode.DoubleRow
    Ident = mybir.ActivationFunctionType.Identity
    NPAIR = 15

    nc = bacc.Bacc("TRN2", target_bir_lowering=False, debug=False)

    zq = nc.dram_tensor("zq", [2, 128, KCH, LP], f8, kind="ExternalInput").ap()
    zr = nc.dram_tensor("zr", [NPAIR, 128, KCH, LP], f8, kind="ExternalInput").ap()
    q2n = nc.dram_tensor("q2n", [2, 128, NQB], f32, kind="ExternalInput").ap()
    onesw = nc.dram_tensor("onesw", [66, 128], f16, kind="ExternalInput").ap()
    rnt = nc.dram_tensor("rnt", [NPAIR, 66, LP], f16, kind="ExternalInput").ap()
    mrow = nc.dram_tensor("mrow", [NPAIR, 128, NQB], f16, kind="ExternalOutput").ap()
    mcol = nc.dram_tensor("mcol", [NPAIR, 128, L], f16, kind="ExternalOutput").ap()

    with tile.TileContext(nc) as tc:
        with (
            tc.tile_pool(name="qpool", bufs=1) as qpool,
            tc.tile_pool(name="refpool", bufs=4) as refpool,
            tc.tile_pool(name="nbpool", bufs=4) as nbpool,
            tc.tile_pool(name="rowpool", bufs=3) as rowpool,
            tc.tile_pool(name="maccpool", bufs=3) as maccpool,
            tc.tile_pool(name="scrqpool", bufs=5) as scrqpool,
            tc.tile_pool(name="smpool", bufs=8) as smpool,
            tc.tile_pool(name="psum", bufs=8, space="PSUM") as psum,
        ):
            qsb = []
            for i in range(2):
                t = qpool.tile([128, KCH, LP], f8, name=f"q{i}", tag=f"q{i}")
                nc.sync.dma_start(t[:], zq[i])
                qsb.append(t)
            q2sb = []
            for i in range(2):
                t = qpool.tile([128, NQB], f32, name=f"q2n{i}", tag=f"q2n{i}")
                nc.sync.dma_start(t[:], q2n[i])
                q2sb.append(t)
            onesb = qpool.tile([66, 128], f16, name="onesb", tag="onesb")
            nc.sync.dma_start(onesb[:], onesw)

            for j in range(NPAIR):
                qi = 0 if j < 8 else 1
                rsb = refpool.tile([128, KCH, LP], f8, name="ref", tag="ref")
                nc.sync.dma_start(rsb[:], zr[j])
                rnsb = nbpool.tile([66, LP], f16, name="rnt", tag="rnt")
                nc.sync.dma_start(rnsb[:], rnt[j])
                mrow_t = rowpool.tile([128, NQB], f16, name="mrt", tag="mrt")
                macc = maccpool.tile([128, L], f16, name="macc", tag="macc")

                for qb in range(NQB):
                    scrq = None
                    if qb > 0:
                        scrq = scrqpool.tile([128, L], f16, name="scrq",
                                             tag="scrq")
                    dst = macc if qb == 0 else scrq
                    # ref-norm matmuls for the 3 chunks: K=2 row-tiles at
                    # partitions 0/32/64 -> concurrent on the PE array
                    pts = []
                    for ci, (r0, w) in enumerate(CHUNKS):
                        pt = psum.tile([128, 512], f32, name="qr", tag="qr")
                        pts.append(pt)
                        bp = 32 * ci
                        nc.tensor.matmul(
                            pt[:, :w],
                            lhsT=onesb[bp:bp + 2, :],
                            rhs=rnsb[bp:bp + 2, r0:r0 + w],
                            start=True,
                            stop=False,
                        )
                    for ci, (r0, w) in enumerate(CHUNKS):
                        pt = pts[ci]
                        for kk in range(KCH // 2):
                            nc.tensor.matmul(
                                pt[:, :w],
                                lhsT=qsb[qi][:, 2 * kk:2 * kk + 2,
                                             qb * 128:(qb + 1) * 128],
                                rhs=rsb[:, 2 * kk:2 * kk + 2, r0:r0 + w],
                                start=False,
                                stop=(kk == KCH // 2 - 1),
                                perf_mode=DR,
                            )
                        # ACT drains PSUM into the fp16 staging row, adding
                        # the per-partition query-norm bias -0.5|q|^2
                        nc.scalar.activation(
                            dst[:, r0:r0 + w], pt[:, :w], Ident,
                            bias=q2sb[qi][:, qb:qb + 1], scale=1.0)
                    # col: one wide merge across query blocks
                    if qb > 0:
                        nc.vector.tensor_tensor(
                            macc[:], macc[:], scrq[:], op=Alu.max)
                    # row: one wide flat reduce
                    nc.vector.tensor_reduce(
                        mrow_t[:, qb:qb + 1], dst[:], axis=AxX, op=Alu.max)

                nc.sync.dma_start(mrow[j], mrow_t[:])
                nc.sync.dma_start(mcol[j], macc[:])
    nc.compile()
    return nc


def _build2():
    """Phase 2: exact rescue. 64 candidate patches (4 per image, chosen by
    phase-1 scores) as M=64 stationary; each core computes the per-ref-image
    min over ITS OWN 2 images' refs, with the cross term at ~2x-fp16
    precision via a 2-term fp16 split (qh*rh + ql*rh) accumulated in PSUM."""
    import concourse.bacc as bacc
    import concourse.tile as tile
    from concourse import mybir

    f16 = mybir.dt.float16
    f32 = mybir.dt.float32
    Alu = mybir.AluOpType
    AxX = mybir.AxisListType.X
    NT = 16   # 2 terms x 8 k-chunks (fp16 qh*rh + ql*rh)

    nc = bacc.Bacc("TRN2", target_bir_lowering=False, debug=False)
    qc = nc.dram_tensor("qc", [128, NT, 64], f16, kind="ExternalInput").ap()
    rh = nc.dram_tensor("rh", [2, 128, KCH, LP], f16, kind="ExternalInput").ap()
    nb2 = nc.dram_tensor("nb2", [2, 128, LP], f32, kind="ExternalInput").ap()
    out = nc.dram_tensor("m2", [2, 64], f32, kind="ExternalOutput").ap()

    with tile.TileContext(nc) as tc:
        with (
            tc.tile_pool(name="p2", bufs=1) as p2,
            tc.tile_pool(name="ref2", bufs=2) as ref2,
            tc.tile_pool(name="sm2", bufs=8) as sm2,
            tc.tile_pool(name="scr2", bufs=4) as scr2,
            tc.tile_pool(name="ps2", bufs=6, space="PSUM") as ps2,
        ):
            qcs = p2.tile([128, NT, 64], f16, name="qcs")
            nc.sync.dma_start(qcs[:], qc[:])
            for pos in range(2):
                # per-chunk ref DMA: first matmuls start after ~1MB, not 5.8MB
                rts = []
                for ci, (r0, w) in enumerate(CHUNKS):
                    rt = ref2.tile([128, KCH, w], f16, name=f"rh{ci}",
                                   tag=f"rh{ci}")
                    nc.sync.dma_start(rt[:], rh[pos][:, :, r0:r0 + w])
                    rts.append(rt)
                nbt = ref2.tile([128, LP], f32, name="nb_t", tag="nb_t")
                nc.sync.dma_start(nbt[:], nb2[pos])

                prev = None
                for ci, (r0, w) in enumerate(CHUNKS):
                    pt = ps2.tile([64, 512], f32, name="qr2", tag="qr2")
                    for t in range(NT):
                        k = t % KCH
                        nc.tensor.matmul(
                            pt[:, :w],
                            lhsT=qcs[:, t, :],
                            rhs=rts[ci][:, k, :w],
                            start=(t == 0),
                            stop=(t == NT - 1),
                        )
                    scr = scr2.tile([64, 512], f32, name="scr_2", tag="scr_2")
                    nc.vector.tensor_tensor(
                        scr[:, :w], pt[:, :w], nbt[:64, r0:r0 + w], op=Alu.add)
                    cm = sm2.tile([64, 1], f32, name="cm2", tag="cm2")
                    nc.vector.tensor_reduce(cm[:], scr[:, :w], axis=AxX, op=Alu.min)
                    if prev is None:
                        prev = cm
                    else:
                        nx = sm2.tile([64, 1], f32, name="nx2", tag="nx2")
                        nc.vector.tensor_tensor(nx[:], prev[:], cm[:], op=Alu.min)
                        prev = nx
                nc.sync.dma_start(out[pos], prev[:])
    nc.compile()
    return nc


def _host_prep(Z):
    import ml_dtypes
    Zp = np.full((N, LP, C), PAD_VAL, dtype=np.float16)
    Zp[:, :L, :] = Z.astype(np.float16)
    # [j, p, k, r] = Zp[j, r, 128k+p]
    zt_all = np.ascontiguousarray(Zp.reshape(N, LP, KCH, 128).transpose(0, 3, 2, 1))
    # fp8 copy for phase 1 (pad value 2.0 is exact in e4m3)
    Zp8 = Zp.astype(ml_dtypes.float8_e4m3)
    zt8_all = np.ascontiguousarray(Zp8.reshape(N, LP, KCH, 128).transpose(0, 3, 2, 1))
    # fp16 residual of the padded refs (pads are exact in fp16 -> residual 0)
    Zp32 = np.zeros((N, LP, C), dtype=np.float32)
    Zp32[:, :L, :] = Z
    Zp32[:, L:, :] = np.float32(PAD_VAL)
    Zlo = (Zp32 - Zp.astype(np.float32)).astype(np.float16)
    zl_all = np.ascontiguousarray(Zlo.reshape(N, LP, KCH, 128).transpose(0, 3, 2, 1))
    nr = (Z.astype(np.float64) ** 2).sum(-1)
    nrp = np.full((N, LP), PAD_NORM)
    nrp[:, :L] = nr
    nrp = nrp.astype(np.float32)
    return zt_all, zt8_all, zl_all, nrp


def _run_with_retry(nc, in_maps, trace, attempts=2):
    """One retry absorbs transient device-state failures (e.g. a poisoned
    exec unit left over from an unrelated crashed run)."""
    import time
    import concourse.bass_utils as bass_utils

    for a in range(attempts):
        try:
            return bass_utils.run_bass_kernel_spmd(
                nc, in_maps, core_ids=list(range(NCORES)), trace=trace)
        except Exception:
            if a == attempts - 1:
                raise
            time.sleep(5)


def _phase1_v2(zt8_all, nrp, trace):
    if "nc" not in _CACHE:
        _CACHE["nc"] = _build()
    nc = _CACHE["nc"]

    in_maps = []
    for c in range(NCORES):
        order = [(2 * c + t) % N for t in range(N)]
        zt_core = np.ascontiguousarray(zt8_all[order])
        nb_core = np.ascontiguousarray(
            np.broadcast_to(0.5 * nrp[order][:, None, :], (N, 128, LP)).astype(np.float32))
        q2_core = np.ascontiguousarray(
            nrp[2 * c:2 * c + 2].reshape(2, NQB, 128).transpose(0, 2, 1))
        in_maps.append({"zt": zt_core, "nb": nb_core, "q2": q2_core})

    res = _run_with_retry(nc, in_maps, trace)
    _CACHE["last_results"] = res

    patch_scores = np.zeros((N, L), dtype=np.float64)
    for c in range(NCORES):
        sc = res.results[c]["scores"]          # [2, 128, NQB]
        flat = sc.transpose(0, 2, 1).reshape(2, LP)   # [2, qb*128+p]
        patch_scores[2 * c:2 * c + 2] = flat[:, :L]
    return patch_scores


def _phase1_v3(zt8_all, nrp, trace):
    if "nc3" not in _CACHE:
        _CACHE["nc3"] = _build3()
    nc3 = _CACHE["nc3"]

    # ref-norm tiles: hi/lo fp16 split of -0.5|r|^2 at partition rows
    # (0,1), (32,33), (64,65) for the three row-tiled K=2 norm matmuls
    nbneg = (-0.5 * nrp).astype(np.float32)               # [N, LP]
    nb_hi = nbneg.astype(np.float16)
    nb_lo = (nbneg - nb_hi.astype(np.float32)).astype(np.float16)
    rn_all = np.zeros((N, 66, LP), dtype=np.float16)
    for bp in (0, 32, 64):
        rn_all[:, bp] = nb_hi
        rn_all[:, bp + 1] = nb_lo
    onesw = np.zeros((66, 128), dtype=np.float16)
    onesw[[0, 1, 32, 33, 64, 65]] = 1.0

    in_maps = []
    for c in range(NCORES):
        pairs = _pairs_for_core(c)
        qimgs = [c, (c + 8) % N]
        rimgs = [t for (_, t) in pairs]
        q2n_core = np.ascontiguousarray(
            (-0.5 * nrp[qimgs]).reshape(2, NQB, 128).transpose(0, 2, 1)
        ).astype(np.float32)
        in_maps.append({
            "zq": np.ascontiguousarray(zt8_all[qimgs]),
            "zr": np.ascontiguousarray(zt8_all[rimgs]),
            "q2n": q2n_core,
            "onesw": onesw,
            "rnt": np.ascontiguousarray(rn_all[rimgs]),
        })

    res = _run_with_retry(nc3, in_maps, trace)
    _CACHE["last_results"] = res

    allm = np.full((N, L, N), np.inf)
    for c in range(NCORES):
        pairs = _pairs_for_core(c)
        mr = res.results[c]["mrow"]            # [15, 128, NQB] f32, -0.5 d^2
        mc = res.results[c]["mcol"]            # [15, 128, LP] f16, -0.5 d^2
        for j, (qi, ri) in enumerate(pairs):
            vrow = mr[j].T.reshape(LP).astype(np.float64)
            d2r = -2.0 * vrow
            allm[qi, :, ri] = np.sqrt(np.maximum(d2r[:L], 1e-12))
            vcol = mc[j].astype(np.float32).max(0).astype(np.float64)
            d2c = -2.0 * vcol
            allm[ri, :, qi] = np.sqrt(np.maximum(d2c[:L], 1e-12))
    return np.partition(allm, 3, axis=-1)[:, :, :4].mean(-1)   # [N, L]


def kernel(Z, cls_tokens):
    Z = np.asarray(Z)
    cls_tokens = np.asarray(cls_tokens)

    zt_all, zt8_all, zl_all, nrp = _host_prep(Z)
    trace = bool(int(os.environ.get("KERNEL_TRACE", "0")))

    if os.environ.get("KERNEL_V", "3") == "3":
        patch_scores = _phase1_v3(zt8_all, nrp, trace)
    else:
        patch_scores = _phase1_v2(zt8_all, nrp, trace)

    img = patch_scores.max(-1)

    if bool(int(os.environ.get("KERNEL_RESCUE", "1"))):
        img = _rescue(Z, patch_scores, zt_all, zl_all, nrp, trace)

    return _host_tail(img, cls_tokens)


def _rescue(Z, patch_scores, zt_all, zl_all, nrp, trace):
    """Phase 2: recompute the top-4 candidate patches per image at ~fp32
    precision on-device (sharded over ref images) and return exact image
    scores."""
    import concourse.bass_utils as bass_utils

    if "nc2" not in _CACHE:
        _CACHE["nc2"] = _build2()
    nc2 = _CACHE["nc2"]

    NT, P = 16, 4
    cand = np.argsort(-patch_scores, axis=-1)[:, :P]     # [16, 4]
    qidx = cand.reshape(-1)                              # m = img*4 + rank
    qimg = np.repeat(np.arange(N), P)
    qf32 = Z[qimg, qidx].astype(np.float32)              # [64, 1024]
    qs = -2.0 * qf32
    qh = qs.astype(np.float16)
    ql = (qs - qh.astype(np.float32)).astype(np.float16)
    # qc[p, t, m]: t 0-7 -> qh chunk t; 8-15 -> ql
    qc = np.zeros((128, NT, 64), dtype=np.float16)
    qc[:, 0:8] = qh.reshape(64, KCH, 128).transpose(2, 1, 0)   # [128, 8, 64]
    qc[:, 8:16] = ql.reshape(64, KCH, 128).transpose(2, 1, 0)

    in_maps2 = []
    for c in range(NCORES):
        sel = [2 * c, 2 * c + 1]
        in_maps2.append({
            "qc": qc,
            "rh": zt_all[sel],
            "nb2": np.ascontiguousarray(
                np.broadcast_to(nrp[sel][:, None, :], (2, 128, LP))),
        })
    res2 = _run_with_retry(nc2, in_maps2, trace)
    _CACHE["last_results2"] = res2

    m2 = np.zeros((64, N))
    for c in range(NCORES):
        m2[:, 2 * c] = res2.results[c]["m2"][0]
        m2[:, 2 * c + 1] = res2.results[c]["m2"][1]

    q2c = (qf32.astype(np.float64) ** 2).sum(-1)
    d2 = np.maximum(m2 + q2c[:, None], 1e-12)
    d = np.sqrt(d2)
    d[np.arange(64), qimg] = np.inf
    cscore = np.sort(d, axis=-1)[:, :4].mean(-1)         # [64]
    return cscore.reshape(N, P).max(-1)


def _host_tail(img, cls_tokens):
    # ---- tiny tail on host (float64) ----
    s = (img - img.min()) / (img.max() - img.min())
    W = cls_tokens.astype(np.float64) @ cls_tokens.astype(np.float64).T
    outs = []
    for k in (1, 2, 3):
        thr = np.sort(W, axis=-1)[:, N - k][:, None]
        Wm = np.where(W >= thr, W, 0.0)
        P = Wm / Wm.sum(-1, keepdims=True)
        outs.append(P @ s)
    return np.stack(outs, -1).mean(-1).astype(np.float32)

